# revision 7
# baseline (speedup 1.0000x reference)
import numpy as np

# nn_ActNet: 3-layer transformer encoder (B,21,256) + 20-step greedy pointer decode.
# Strategy: pure data parallelism over batch (8192 -> 8 x 1024) for the Bass
# kernel computing the pairwise-distance tensor on all 8 NeuronCores; the
# encoder + decode run in JAX on the neuron device (BatchNorm needs full-batch
# statistics, so that phase is kept numerically identical to the reference).
# The decode avoids argmax/gather primitives (unsupported by neuronx-cc):
# greedy selection uses max + iota + min-index, gathers use one-hot reductions.
B, N, E, M, DK, A, FD = 8192, 21, 256, 8, 32, 3, 3
NCORES = 8
BC = B // NCORES
CCLIP = 10.0
NEG = -1e9
EPS = 1e-5
SCALE = DK ** 0.5

_CACHE = {}


def _build_dis_bass():
    """Bass SPMD kernel: per-core (BC,21,2) coords -> (BC,21*21) distances.

    Layout: batch on partitions (128/tile, BC/128 tiles per core); each lane
    computes its own 21x21 distance matrix with DVE elementwise ops + ACT sqrt.
    dis[b,i,j] = ||xy[b,j]-xy[b,i]||, zeroed for j < A.
    """
    import concourse.bass as bass
    import concourse.mybir as mybir
    import concourse.tile as tile

    nc = bass.Bass()
    xy_in = nc.declare_dram_parameter("xy", [BC, N * 2], mybir.dt.float32, isOutput=False)
    dis_out = nc.declare_dram_parameter("dis", [BC, N * N], mybir.dt.float32, isOutput=True)
    P = 128
    ntiles = BC // P
    xy_v = xy_in.rearrange("(t p) f -> t p f", p=P)
    dis_v = dis_out.rearrange("(t p) f -> t p f", p=P)

    with tile.TileContext(nc) as tc:
        with tc.tile_pool(name="work", bufs=8) as pool:
            for t in range(ntiles):
                xy_flat = pool.tile([P, N * 2], mybir.dt.float32)
                nc.gpsimd.dma_start(out=xy_flat, in_=xy_v[t])
                xy = xy_flat.rearrange("p (n c) -> p n c", c=2)
                x_i = xy[:, :, 0].unsqueeze(2).broadcast_to([P, N, N])
                x_j = xy[:, :, 0].unsqueeze(1).broadcast_to([P, N, N])
                y_i = xy[:, :, 1].unsqueeze(2).broadcast_to([P, N, N])
                y_j = xy[:, :, 1].unsqueeze(1).broadcast_to([P, N, N])
                d2 = pool.tile([P, N, N], mybir.dt.float32)
                tmp = pool.tile([P, N, N], mybir.dt.float32)
                nc.vector.tensor_sub(out=d2, in0=x_j, in1=x_i)
                nc.vector.tensor_mul(out=d2, in0=d2, in1=d2)
                nc.vector.tensor_sub(out=tmp, in0=y_j, in1=y_i)
                nc.vector.tensor_mul(out=tmp, in0=tmp, in1=tmp)
                nc.vector.tensor_add(out=d2, in0=d2, in1=tmp)
                dis_t = pool.tile([P, N * N], mybir.dt.float32)
                nc.scalar.sqrt(out=dis_t, in_=d2.rearrange("p n k -> p (n k)"))
                nc.vector.memset(dis_t.rearrange("p (n k) -> p n k", k=N)[:, :, 0:A], 0.0)
                nc.gpsimd.dma_start(out=dis_v[t], in_=dis_t)
    return nc


def _dis_on_device(x_np):
    from concourse.bass_utils import run_bass_kernel_spmd

    if "nc" not in _CACHE:
        _CACHE["nc"] = _build_dis_bass()
    nc = _CACHE["nc"]
    xy = np.ascontiguousarray(x_np[:, :, :2].reshape(B, N * 2).astype(np.float32))
    in_maps = [{"xy": xy[c * BC:(c + 1) * BC]} for c in range(NCORES)]
    res = run_bass_kernel_spmd(nc, in_maps, list(range(NCORES)))
    shards = [np.asarray(res.results[c]["dis"]).reshape(BC, N, N) for c in range(NCORES)]
    return np.concatenate(shards, axis=0)


def _make_fns():
    import jax
    import jax.numpy as jnp

    def _lin(p, x):
        return x @ p[0] + p[1]

    def _bn(x, p):
        g, b = p
        m = x.mean(axis=(0, 1))
        v = jnp.var(x, axis=(0, 1))
        return g * (x - m) * jax.lax.rsqrt(v + EPS) + b

    def _mha(x, pq, pk, pv, pw):
        b = x.shape[0]
        q = _lin(pq, x).reshape(b, N, M, DK)
        k = _lin(pk, x).reshape(b, N, M, DK)
        v = _lin(pv, x).reshape(b, N, M, DK)
        s = jnp.einsum('bnmd,bkmd->bmnk', q, k) / SCALE
        a = jax.nn.softmax(s, axis=-1)
        z = jnp.einsum('bmnk,bkmd->bnmd', a, v).reshape(b, N, E)
        return _lin(pw, z) + x

    def encoder(x_, params):
        x = _lin(params['embedding'], x_)
        for l in ('1', '2', '3'):
            x = _mha(x, params['wq' + l], params['wk' + l], params['wv' + l], params['w' + l])
            x = _bn(x, params['bn' + l + '1'])
            h = jax.nn.relu(_lin(params['ffc' + l + '1'], x))
            x = _lin(params['ffc' + l + '2'], h) + x
            x = _bn(x, params['bn' + l + '2'])
        ave = x.mean(axis=1)
        k4 = _lin(params['wk4'], x).reshape(x.shape[0], N, M, DK)
        v4 = _lin(params['wv4'], x).reshape(x.shape[0], N, M, DK)
        k5 = _lin(params['wk5'], x)
        return x, ave, k4, v4, k5

    def step(mask, oh_idx, distance, x, ave, k4, v4, k5, dis, params):
        b = x.shape[0]
        iota = jnp.arange(N, dtype=jnp.int32)[None, :]  # (1,N)
        mask = mask | (oh_idx > 0.5)
        now = jnp.einsum('bn,bne->be', oh_idx, x)
        gi = jnp.concatenate([ave, now], axis=-1)
        q = _lin(params['wq4'], gi).reshape(b, M, DK)
        s = jnp.einsum('bmd,bnmd->bmn', q, k4) / SCALE
        s = jnp.where(mask[:, None, :], NEG, s)
        a = jax.nn.softmax(s, axis=-1)
        z = jnp.einsum('bmn,bnmd->bmd', a, v4).reshape(b, E)
        z = _lin(params['w4'], z)
        q5 = _lin(params['wq5'], z)
        logits = jnp.tanh(jnp.einsum('be,bne->bn', q5, k5) / SCALE) * CCLIP
        logits = jnp.where(mask, NEG, logits)
        p = jax.nn.softmax(logits, axis=-1)
        pmax = jnp.max(p, axis=-1, keepdims=True)
        is_max = p >= pmax
        nidx = jnp.min(jnp.where(is_max, iota, N), axis=-1).astype(jnp.int32)
        oh_nidx = (iota == nidx[:, None]).astype(jnp.float32)
        pro_i = jnp.sum(p * oh_nidx, axis=-1)
        dis_row = jnp.einsum('bn,bnk->bk', oh_idx, dis)
        distance = distance + jnp.sum(dis_row * oh_nidx, axis=-1)
        return mask, oh_nidx, distance, nidx, pro_i

    return jax.jit(encoder), jax.jit(step)


def _decode_loop(step, x, ave, k4, v4, k5, dis, params):
    import jax.numpy as jnp

    b = x.shape[0]
    mask = jnp.zeros((b, N), bool)
    oh_idx = jnp.zeros((b, N), jnp.float32).at[:, 0].set(1.0)
    distance = jnp.zeros((b,), jnp.float32)
    seqs, pros = [], []
    for _ in range(N - 1):
        mask, oh_idx, distance, nidx, pro_i = step(
            mask, oh_idx, distance, x, ave, k4, v4, k5, dis, params)
        seqs.append(nidx)
        pros.append(pro_i)
    seq = jnp.stack(seqs, axis=1).astype(jnp.float32)
    pro = jnp.stack(pros, axis=1)
    return seq, pro, distance


def _dis_numpy(x_np):
    xy = x_np[:, :, :2]
    diff = xy[:, None, :, :] - xy[:, :, None, :]
    dis = np.sqrt(np.sum(diff * diff, axis=-1))
    dis[:, :, :A] = 0.0
    return dis.astype(np.float32)


def kernel(x_, params, is_train):
    import jax

    x_np = np.asarray(x_, dtype=np.float32)
    try:
        dis = _dis_on_device(x_np)
    except Exception as e:
        print(f"kernel: bass dis kernel failed ({type(e).__name__}); numpy fallback")
        dis = _dis_numpy(x_np)

    if "fns" not in _CACHE:
        _CACHE["fns"] = _make_fns()
    encoder, step = _CACHE["fns"]
    params_h = jax.tree_util.tree_map(np.asarray, params)

    def run_on(dev):
        x_d = jax.device_put(x_np, dev)
        dis_d = jax.device_put(dis, dev)
        params_d = jax.tree_util.tree_map(lambda a: jax.device_put(a, dev), params_h)
        enc = encoder(x_d, params_d)
        out = _decode_loop(step, *enc, dis_d, params_d)
        return tuple(np.asarray(o) for o in out)

    try:
        return run_on(jax.devices()[0])
    except Exception as e:  # neuron compile failure -> CPU fallback
        print(f"kernel: neuron path failed ({type(e).__name__}: {e}); CPU fallback")
        return run_on(jax.devices("cpu")[0])
